# revision 48
# baseline (speedup 1.0000x reference)
# Multi-head attention (B=2, S=2048, D=1024, H=16, dh=64) on 8 TRN2 NeuronCores.
# Sharding: core = batch * 4 + head_group; each core handles one batch and 4
# heads. Host prep: q/k/v pre-tiled to [t, p, c, s] bf16 so each per-t DMA
# reads 8KB contiguous per partition (few, large DMA descriptors); weight
# slices likewise partition-major; bf16 partial outputs summed on host.
#
# Schedule: window W0 = projections for token-tile 0 (loads staged: q/k
# dependencies stream at full HBM bandwidth first; v/keep/wo and later x tiles
# are released by "pacer" ops on the gpsimd queue so they never steal
# bandwidth from the startup-critical path). Window W(t+1) runs attention for
# q-tile t (causal => only needs k-tiles <= 4t+3) with scores software-
# pipelined one k-tile ahead of PV, and with independent PE work — projection
# chains for t+1, deferred normalization + output projection of earlier
# q-tiles — interleaved into the tile stream as fill so the PE never waits on
# the ScalarE exp round trip.
#
# Attention tile: scoresT = K^T@Q (2-head row-packed pair of K=64 matmuls at
# tile_position (0,0)/(64,0), shared 2-bank psum tile) -> one exp per
# head-pair (ScalarE, 1/8 scale fused) -> multiplicative keep-mask on partial
# tiles only (DVE) -> PV with ones-augmented V (M=65) giving the softmax
# denominators for free in psum row 64 -> unnormalized att evacuated to SBUF;
# normalization (reciprocal + K=1 broadcast matmuls + in-place multiply) and
# the row-parallel Wo projection run as fill units one window later.
import numpy as np
import ml_dtypes

import concourse.bass as bass
import concourse.tile as tile
from concourse import bacc, mybir
from concourse import bass_utils

B, S, D = 2, 2048, 1024
H, DH = 16, 64
NCORES = 8
GROUPS = 4            # head groups per batch (cores per batch)
HPG = 4               # heads per group
FPG = HPG * DH        # 256 features per group
SQ_T, SK_T = 512, 128
NSQ, NSK = S // SQ_T, S // SK_T
NCH = D // 128        # 8 contraction chunks of d_model
BF16 = ml_dtypes.bfloat16

_BUILT = {}


def _classify(mask):
    """Per-tile mask classification in scoresT space: tile (i, j) covers
    k in [i*128, (i+1)*128), q in [j*512, (j+1)*512)."""
    keep_t = (~np.asarray(mask, dtype=bool)).T  # [k, q], True = attend
    cls = {}
    ptiles = []
    for j in range(NSQ):
        for i in range(NSK):
            sub = keep_t[i * SK_T:(i + 1) * SK_T, j * SQ_T:(j + 1) * SQ_T]
            if not sub.any():
                cls[(i, j)] = "skip"
            elif sub.all():
                cls[(i, j)] = ("full", 0, SQ_T)
            else:
                # column bounding range with any unmasked entry
                cols = np.flatnonzero(sub.any(axis=0))
                cls[(i, j)] = (len(ptiles), int(cols[0]), int(cols[-1]) + 1)
                ptiles.append(np.ascontiguousarray(sub.astype(BF16)))
    return cls, ptiles


def _build(cls, n_ptiles):
    nc = bacc.Bacc("TRN2", target_bir_lowering=False, debug=False)
    dt = mybir.dt
    f32, bf = dt.float32, dt.bfloat16
    EXP = mybir.ActivationFunctionType.Exp
    NP_ = max(n_ptiles, 1)

    xq = nc.dram_tensor("xqt", [NSQ, 128, NCH, SQ_T], bf,
                        kind="ExternalInput").ap()
    xk = nc.dram_tensor("xkt", [NSQ, 128, NCH, SQ_T], bf,
                        kind="ExternalInput").ap()
    xv = nc.dram_tensor("xvt", [NSQ, 128, NCH, SQ_T], bf,
                        kind="ExternalInput").ap()
    wq = nc.dram_tensor("wqt", [128, NCH, FPG], bf, kind="ExternalInput").ap()
    wk = nc.dram_tensor("wkt", [128, NCH, FPG], bf, kind="ExternalInput").ap()
    wv = nc.dram_tensor("wvt", [128, NCH, FPG], bf, kind="ExternalInput").ap()
    wo = nc.dram_tensor("wot", [128, FPG // 128, D], bf,
                        kind="ExternalInput").ap()
    kp = nc.dram_tensor("keep", [128, NP_, SQ_T], bf,
                        kind="ExternalInput").ap()
    out = nc.dram_tensor("out", [S, D], bf, kind="ExternalOutput").ap()

    out_v = out.rearrange("(r p) o -> r p o", p=128)

    # per-j contiguous partial-tile ranges in the packed keep tensor
    kranges = []
    lo = 0
    for j in range(NSQ):
        hi = lo
        for i in range(NSK):
            cj = cls[(i, j)]
            if cj != "skip" and cj[0] != "full":
                hi = cj[0] + 1
        kranges.append((lo, max(hi, lo)))
        lo = max(hi, lo)

    with tile.TileContext(nc) as tc:
        with (
            tc.tile_pool(name="consts", bufs=1) as consts,
            tc.tile_pool(name="x", bufs=2) as xpool,
            tc.tile_pool(name="sc", bufs=2, space="PSUM") as sc_ps,
            tc.tile_pool(name="pv", bufs=2, space="PSUM") as pv_ps,
            tc.tile_pool(name="aux", bufs=2, space="PSUM") as aux_ps,
            tc.tile_pool(name="work", bufs=4) as work,
            tc.tile_pool(name="probs", bufs=8) as prpool,
        ):
            wq_sb = consts.tile([128, NCH, FPG], bf)
            wk_sb = consts.tile([128, NCH, FPG], bf)
            wv_sb = consts.tile([128, NCH, FPG], bf)
            wo_sb = consts.tile([128, FPG // 128, D], bf)
            keep_sb = consts.tile([128, NP_, SQ_T], bf)
            ones_sb = consts.tile([128, 128], bf)
            qh_sb = consts.tile([128, 2, S], bf)
            kh_sb = consts.tile([128, 2, S], bf)
            vh_sb = consts.tile([128, NSK, HPG, DH + 1], bf)
            att_sb = consts.tile([128, 2, S], bf)
            # per-(j%2, hp) normalization scratch; denominator rows 0 and 32
            l4 = [[consts.tile([128, SQ_T], f32, name=f"l4_{p}{h}")
                   for h in range(2)] for p in range(2)]
            r4f = [[consts.tile([128, SQ_T], f32, name=f"r4f_{p}{h}")
                    for h in range(2)] for p in range(2)]
            r4 = [[consts.tile([128, SQ_T], bf, name=f"r4_{p}{h}")
                   for h in range(2)] for p in range(2)]

            xtiles = {}
            pace_sb = consts.tile([1, 16], bf)

            def load_x(t):
                """Emit DMA triggers for iteration t's x tiles. For t=0 these
                go serially on sync in priority order; for t>0 the caller has
                placed a pacer on the gpsimd queue so the loads don't compete
                with the startup-critical transfers."""
                xq_t = xpool.tile([128, NCH, SQ_T], bf, tag="xq", name="xq_t")
                xk_t = xpool.tile([128, NCH, SQ_T], bf, tag="xk", name="xk_t")
                xv_t = xpool.tile([128, NCH, SQ_T], bf, tag="xv", name="xv_t")
                klo, khi = kranges[t]
                if t == 0:
                    # stage 1a: ONLY the q dependencies in flight, at full
                    # HBM bandwidth; k then v/keep/wo stages are released by
                    # pacers (load_stage1k / load_stage2) as q data lands
                    nc.sync.dma_start(wq_sb[:, 0:1, :], wq[:, 0:1, :])
                    nc.sync.dma_start(xq_t[:, 0:1, :], xq[t, :, 0:1, :])
                    nc.sync.dma_start(wq_sb[:, 1:2, :], wq[:, 1:2, :])
                    nc.sync.dma_start(xq_t[:, 1:2, :], xq[t, :, 1:2, :])
                    nc.sync.dma_start(wq_sb[:, 2:4, :], wq[:, 2:4, :])
                    nc.sync.dma_start(xq_t[:, 2:4, :], xq[t, :, 2:4, :])
                    nc.sync.dma_start(wq_sb[:, 4:8, :], wq[:, 4:8, :])
                    nc.sync.dma_start(xq_t[:, 4:8, :], xq[t, :, 4:8, :])
                    nc.gpsimd.memset(ones_sb[:], 1.0)
                    nc.gpsimd.memset(vh_sb[:], 1.0)  # ones col DH survives
                    for p in range(2):               # unused part. -> r = 1
                        nc.gpsimd.memset(l4[p][0][:], 1.0)
                        nc.gpsimd.memset(l4[p][1][:], 1.0)
                else:
                    nc.gpsimd.dma_start(xq_t[:], xq[t])
                    nc.gpsimd.dma_start(xk_t[:], xk[t])
                    nc.gpsimd.dma_start(xv_t[:], xv[t])
                    if khi > klo:
                        nc.gpsimd.dma_start(keep_sb[:, klo:khi, :],
                                            kp[:, klo:khi, :])
                xtiles[t] = (xq_t, xk_t, xv_t)

            def load_stage1k():
                """Release t=0's k loads once the q transfers have landed
                (pacer on the last xq piece), so q streams at full BW."""
                xk_t = xtiles[0][1]
                nc.gpsimd.tensor_copy(pace_sb[:], xtiles[0][0][0:1, 7, 0:16])
                nc.gpsimd.dma_start(wk_sb[:], wk[:])
                nc.gpsimd.dma_start(xk_t[:], xk[0])

            def load_stage2():
                """Release t=0's v/keep/wo loads once the first q-chain has
                evacuated (pacer), keeping startup bandwidth for q/k."""
                xv_t = xtiles[0][2]
                klo, khi = kranges[0]
                nc.gpsimd.tensor_copy(pace_sb[:],
                                      qh_sb[0:1, 0, bass.ds(0, 16)])
                nc.gpsimd.dma_start(wv_sb[:], wv[:])
                nc.gpsimd.dma_start(xv_t[:], xv[0])
                if khi > klo:
                    nc.gpsimd.dma_start(keep_sb[:, klo:khi, :],
                                        kp[:, klo:khi, :])
                nc.gpsimd.dma_start(wo_sb[:], wo[:])

            def proj_units(t):
                """Independent PE chain units for token-tile t's projections,
                in dependency order (q chains, k chains, v chains)."""
                xq_t, xk_t, xv_t = xtiles[t]
                sl = bass.ts(t, SQ_T)
                units = []
                for hout_, hp_ in ((qh_sb, 0), (qh_sb, 1), (kh_sb, 0),
                                   (kh_sb, 1)):
                    def u(hout=hout_, hp=hp_):
                        wsb = wq_sb if hout is qh_sb else wk_sb
                        xin = xq_t if hout is qh_sb else xk_t
                        hsl = bass.ts(hp, 128)
                        ps = aux_ps.tile([128, SQ_T], f32, tag="aux",
                                         name="ps")
                        for c in range(NCH):
                            nc.tensor.matmul(ps[:], wsb[:, c, hsl],
                                             xin[:, c, :], start=(c == 0),
                                             stop=(c == NCH - 1))
                        nc.vector.tensor_copy(hout[:, hp, sl], ps[:])
                    units.append(u)
                for s4_ in range(SQ_T // SK_T):
                    def u(s4=s4_):
                        i = t * (SQ_T // SK_T) + s4
                        psv = aux_ps.tile([128, SQ_T], f32, tag="aux",
                                          name="psv")
                        for c in range(NCH):
                            nc.tensor.matmul(psv[:, 0:FPG],
                                             xv_t[:, c, bass.ts(s4, SK_T)],
                                             wv_sb[:, c, :],
                                             start=(c == 0),
                                             stop=(c == NCH - 1))
                        nc.vector.tensor_copy(
                            vh_sb[:, i, :, 0:DH],
                            psv[:, 0:FPG].rearrange("p (h d) -> p h d",
                                                    h=HPG))
                    units.append(u)
                return units

            def attn_core(j, fill, fill_late=(), fill_after=(),
                          after_hp0=None):
                """Scores/exp/mask/PV for q-tile j, both head pairs, with
                independent `fill` units interleaved between a tile's scores
                and its PV so PE never waits on ScalarE exp. `fill_late` units
                are spread over the second (hp=1) half only; `after_hp0` runs
                right after hp=0's PSUM evacuation. Leaves unnormalized att in
                att_sb and denominators in l4[j % 2]."""
                jsl = bass.ts(j, SQ_T)
                kept = [i for i in range(NSK) if cls[(i, j)] != "skip"]
                ntiles = 2 * len(kept)
                fill = list(fill)
                # spread fill units evenly across the tile stream; late units
                # over the second half only
                fill_at = {}
                for f_i in range(len(fill)):
                    fill_at.setdefault(f_i * ntiles // max(len(fill), 1),
                                       []).append(fill[f_i])
                fill_late = list(fill_late)
                half = ntiles // 2
                for f_i in range(len(fill_late)):
                    pos = half + f_i * (ntiles - half) // max(len(fill_late),
                                                              1)
                    fill_at.setdefault(pos, []).append(fill_late[f_i])
                tile_no = 0
                for hp in range(2):
                    pv0 = pv_ps.tile([DH + 1, SQ_T], f32, tag="pv",
                                     name="pv0")
                    pv1 = pv_ps.tile([DH + 1, SQ_T], f32, tag="pv",
                                     name="pv1")

                    def tile_rng(n):
                        c, c0, c1 = cls[(kept[n], j)]
                        if n == 0:
                            c0, c1 = 0, SQ_T  # first tile must cover the bank
                        return c, c0, c1

                    def emit_scores(n):
                        c, c0, c1 = tile_rng(n)
                        isl = bass.ts(kept[n], SK_T)
                        qsl = bass.ds(j * SQ_T + c0, c1 - c0)
                        sc = sc_ps.tile([128, 2, SQ_T], f32, tag="sc",
                                        name="sc")
                        nc.tensor.matmul(sc[:, 0, c0:c1], kh_sb[0:64, hp, isl],
                                         qh_sb[0:64, hp, qsl], start=True,
                                         stop=True, tile_position=(0, 0))
                        nc.tensor.matmul(sc[:, 1, c0:c1],
                                         kh_sb[64:128, hp, isl],
                                         qh_sb[64:128, hp, qsl], start=True,
                                         stop=True, tile_position=(64, 0))
                        return sc

                    sc_n = emit_scores(0)
                    for n, i in enumerate(kept):
                        sc, (c, c0, c1) = sc_n, tile_rng(n)
                        if n + 1 < len(kept):
                            sc_n = emit_scores(n + 1)
                        for f in fill_at.pop(tile_no, ()):
                            f()
                        tile_no += 1
                        pr = prpool.tile([128, 2, SQ_T], bf, tag="probs",
                                         name="pr")
                        nc.scalar.activation(pr[:, :, c0:c1], sc[:, :, c0:c1],
                                             EXP, scale=0.125)
                        if c != "full":
                            nc.vector.tensor_mul(pr[:, 0, c0:c1],
                                                 pr[:, 0, c0:c1],
                                                 keep_sb[:, c, c0:c1])
                            nc.vector.tensor_mul(pr[:, 1, c0:c1],
                                                 pr[:, 1, c0:c1],
                                                 keep_sb[:, c, c0:c1])
                        nc.tensor.matmul(pv0[:, c0:c1],
                                         vh_sb[:, i, 2 * hp + 0, :],
                                         pr[:, 0, c0:c1], start=(n == 0),
                                         stop=(n == len(kept) - 1))
                        nc.tensor.matmul(pv1[:, c0:c1],
                                         vh_sb[:, i, 2 * hp + 1, :],
                                         pr[:, 1, c0:c1], start=(n == 0),
                                         stop=(n == len(kept) - 1))
                    # evacuate psum fast: unnormalized att + denominators.
                    # On the final head-pair the exp stream is finished, so
                    # the shift-free att block can evacuate on idle ScalarE
                    # and the denominator rows go first, shortening the tail
                    # DVE chain that gates the last normalization.
                    lj = l4[j % 2][hp]
                    if j == NSQ - 1 and hp == 1:
                        nc.vector.tensor_copy(lj[0:1, :], pv0[DH:DH + 1, :])
                        nc.scalar.copy(att_sb[0:64, hp, jsl], pv0[0:64, :])
                        nc.vector.tensor_copy(lj[32:33, :],
                                              pv1[DH:DH + 1, :])
                        nc.vector.tensor_copy(att_sb[64:128, hp, jsl],
                                              pv1[0:64, :])
                    else:
                        nc.vector.tensor_copy(lj[0:1, :], pv0[DH:DH + 1, :])
                        nc.vector.tensor_copy(att_sb[0:64, hp, jsl],
                                              pv0[0:64, :])
                        nc.vector.tensor_copy(lj[32:33, :],
                                              pv1[DH:DH + 1, :])
                        nc.vector.tensor_copy(att_sb[64:128, hp, jsl],
                                              pv1[0:64, :])
                    if hp == 0 and after_hp0 is not None:
                        after_hp0()
                for fs in fill_at.values():
                    for f in fs:
                        f()
                for f in fill_after:
                    f()

            def norm_hp_unit(j, hp, tail=False):
                """Normalization of q-tile j's head pair hp: r = 1/l on the
                pair's denominator rows, K=1 broadcast matmuls, in-place
                multiply of att_sb. In tail mode the shift-free copies run
                on ScalarE (idle once the exp stream ends)."""
                cp = nc.scalar.copy if tail else nc.vector.tensor_copy

                def u():
                    jsl = bass.ts(j, SQ_T)
                    lj = l4[j % 2][hp]
                    rjf = r4f[j % 2][hp]
                    rj = r4[j % 2][hp]
                    nc.vector.reciprocal_approx_fast(rjf[:], lj[:])
                    cp(rj[0:33, :], rjf[0:33, :])
                    rb_sb = work.tile([128, 2, SQ_T], bf, tag="rbsb",
                                      name="rb_sb")
                    rb = aux_ps.tile([128, SQ_T], f32, tag="aux", name="rb")
                    nc.tensor.matmul(rb[0:64, :], ones_sb[0:1, 0:64],
                                     rj[0:1, :], start=True,
                                     stop=True, tile_position=(0, 0))
                    nc.tensor.matmul(rb[64:128, :],
                                     ones_sb[32:33, 64:128],
                                     rj[32:33, :], start=True,
                                     stop=True, tile_position=(32, 64))
                    cp(rb_sb[:, hp, :], rb[:])
                    nc.vector.tensor_mul(att_sb[:, hp, jsl],
                                         att_sb[:, hp, jsl],
                                         rb_sb[:, hp, :])
                return u

            def outproj_units(j):
                """Row-parallel Wo for q-tile j as independent units (requires
                both of j's norm_hp units emitted first)."""
                units = []
                for t4_ in range(SQ_T // 128):
                    def u(t4=t4_):
                        r_ = j * (SQ_T // 128) + t4
                        tsl = bass.ds(j * SQ_T + t4 * 128, 128)
                        ost = work.tile([128, D], bf, tag="ost", name="ost")
                        for o in range(2):
                            po = aux_ps.tile([128, SQ_T], f32, tag="aux",
                                             name="po")
                            for hp in range(2):
                                nc.tensor.matmul(po[:], att_sb[:, hp, tsl],
                                                 wo_sb[:, hp,
                                                       bass.ts(o, 512)],
                                                 start=(hp == 0),
                                                 stop=(hp == 1))
                            nc.vector.tensor_copy(ost[:, bass.ts(o, 512)],
                                                  po[:])
                        nc.sync.dma_start(out_v[r_], ost[:])
                    units.append(u)
                return units

            # Window schedule: W0 = proj(0); W(t+1) = attn(t) interleaved
            # with proj(t+1) chains and norm/outproj(t-1) as fill. The last
            # window back-loads its fill, runs norm(hp=0) right after hp=0's
            # evacuation, and leaves only norm(hp=1) + outproj for the tail.
            load_x(0)
            load_stage1k()
            p0_units = proj_units(0)
            p0_units[0]()
            load_stage2()
            for u in p0_units[1:]:
                u()
            nc.gpsimd.tensor_copy(pace_sb[:], qh_sb[0:1, 0, bass.ds(0, 16)])
            load_x(1)
            last = NSQ - 1
            fills = {
                0: lambda: proj_units(1),
                1: lambda: proj_units(2) + [norm_hp_unit(0, 0),
                                            norm_hp_unit(0, 1)],
                2: lambda: (proj_units(3) + [norm_hp_unit(1, 0),
                                             norm_hp_unit(1, 1)]
                            + outproj_units(0)),
                3: lambda: ([norm_hp_unit(2, 0), norm_hp_unit(2, 1)]
                            + outproj_units(1) + outproj_units(2)),
            }
            for t in range(NSQ):
                fill = fills[t]()
                fill_late = [norm_hp_unit(t, 0)] if t == last else []
                attn_core(t, fill, fill_late, [])
                if t + 2 < NSQ:
                    # pacer: hold iteration t+2's loads off the DMA engines
                    # until iteration t+1's q-projection has landed, so
                    # startup-critical transfers get full HBM bandwidth
                    nc.gpsimd.tensor_copy(
                        pace_sb[:],
                        qh_sb[0:1, 0, bass.ds((t + 1) * SQ_T, 16)])
                    load_x(t + 2)
            norm_hp_unit(last, 1, tail=True)()
            for u in outproj_units(last):
                u()

    nc.compile()
    return nc


def _get_nc(mask):
    key = hash(np.asarray(mask, dtype=bool).tobytes())
    if key not in _BUILT:
        cls, ptiles = _classify(mask)
        _BUILT[key] = (_build(cls, len(ptiles)), cls, ptiles)
    return _BUILT[key]


def _tile_x(x):
    """[S, D] f32 -> [NSQ, 128, NCH, SQ_T] bf16, contiguous per partition."""
    xt = x.T.astype(BF16)                      # [D, S]
    xt = xt.reshape(NCH, 128, NSQ, SQ_T)       # [c, p, t, s]
    return np.ascontiguousarray(xt.transpose(2, 1, 0, 3))


def _tile_w(w):
    """[FPG, D] slice (already W[fsl,:].T = [D, FPG]) -> [128, NCH, FPG]."""
    return np.ascontiguousarray(w.reshape(NCH, 128, FPG).transpose(1, 0, 2))


def _kernel_impl(q, k, v, attn_mask, Wq, Wk, Wv, Wo, trace=False):
    q = np.asarray(q, dtype=np.float32)
    k = np.asarray(k, dtype=np.float32)
    v = np.asarray(v, dtype=np.float32)
    nc, cls, ptiles = _get_nc(attn_mask)

    if ptiles:
        keep_packed = np.stack(ptiles, axis=0)          # [n, 128, 512]
    else:
        keep_packed = np.zeros((1, SK_T, SQ_T), dtype=BF16)
    keep_packed = np.ascontiguousarray(keep_packed.transpose(1, 0, 2))

    xt = {}
    for b in range(B):
        xt[("q", b)] = _tile_x(q[b])
        xt[("k", b)] = _tile_x(k[b])
        xt[("v", b)] = _tile_x(v[b])
    wslices = {}
    for g in range(GROUPS):
        fsl = slice(g * FPG, (g + 1) * FPG)
        wslices[("wq", g)] = _tile_w(Wq[fsl, :].T.astype(BF16))
        wslices[("wk", g)] = _tile_w(Wk[fsl, :].T.astype(BF16))
        wslices[("wv", g)] = _tile_w(Wv[fsl, :].T.astype(BF16))
        wot = Wo[:, fsl].T.astype(BF16)                 # [FPG, D]
        wslices[("wo", g)] = np.ascontiguousarray(
            wot.reshape(2, 128, D).transpose(1, 0, 2))
    in_maps = []
    for core in range(NCORES):
        b, g = core // GROUPS, core % GROUPS
        in_maps.append({
            "xqt": xt[("q", b)], "xkt": xt[("k", b)], "xvt": xt[("v", b)],
            "wqt": wslices[("wq", g)], "wkt": wslices[("wk", g)],
            "wvt": wslices[("wv", g)], "wot": wslices[("wo", g)],
            "keep": keep_packed,
        })

    res = bass_utils.run_bass_kernel_spmd(
        nc, in_maps, core_ids=list(range(NCORES)), trace=trace)

    out = np.zeros((B, S, D), dtype=np.float32)
    for core in range(NCORES):
        out[core // GROUPS] += res.results[core]["out"].astype(np.float32)
    return out, res


def kernel(q, k, v, attn_mask, Wq, Wk, Wv, Wo):
    out, _ = _kernel_impl(q, k, v, attn_mask, Wq, Wk, Wv, Wo)
    return out


# revision 49
# speedup vs baseline: 1.0242x; 1.0242x over previous
# Multi-head attention (B=2, S=2048, D=1024, H=16, dh=64) on 8 TRN2 NeuronCores.
# Sharding: core = batch * 4 + head_group; each core handles one batch and 4
# heads. Host prep: q/k/v pre-tiled to [t, p, c, s] bf16 so each per-t DMA
# reads 8KB contiguous per partition (few, large DMA descriptors); weight
# slices likewise partition-major; bf16 partial outputs summed on host.
#
# Schedule: window W0 = projections for token-tile 0 (loads staged: q/k
# dependencies stream at full HBM bandwidth first; v/keep/wo and later x tiles
# are released by "pacer" ops on the gpsimd queue so they never steal
# bandwidth from the startup-critical path). Window W(t+1) runs attention for
# q-tile t (causal => only needs k-tiles <= 4t+3) with scores software-
# pipelined one k-tile ahead of PV, and with independent PE work — projection
# chains for t+1, deferred normalization + output projection of earlier
# q-tiles — interleaved into the tile stream as fill so the PE never waits on
# the ScalarE exp round trip.
#
# Attention tile: scoresT = K^T@Q (2-head row-packed pair of K=64 matmuls at
# tile_position (0,0)/(64,0), shared 2-bank psum tile) -> one exp per
# head-pair (ScalarE, 1/8 scale fused) -> multiplicative keep-mask on partial
# tiles only (DVE) -> PV with ones-augmented V (M=65) giving the softmax
# denominators for free in psum row 64 -> unnormalized att evacuated to SBUF;
# normalization (reciprocal + K=1 broadcast matmuls + in-place multiply) and
# the row-parallel Wo projection run as fill units one window later.
import numpy as np
import ml_dtypes

import concourse.bass as bass
import concourse.tile as tile
from concourse import bacc, mybir
from concourse import bass_utils

B, S, D = 2, 2048, 1024
H, DH = 16, 64
NCORES = 8
GROUPS = 4            # head groups per batch (cores per batch)
HPG = 4               # heads per group
FPG = HPG * DH        # 256 features per group
SQ_T, SK_T = 512, 128
NSQ, NSK = S // SQ_T, S // SK_T
NCH = D // 128        # 8 contraction chunks of d_model
BF16 = ml_dtypes.bfloat16

_BUILT = {}


def _classify(mask):
    """Per-tile mask classification in scoresT space: tile (i, j) covers
    k in [i*128, (i+1)*128), q in [j*512, (j+1)*512)."""
    keep_t = (~np.asarray(mask, dtype=bool)).T  # [k, q], True = attend
    cls = {}
    ptiles = []
    for j in range(NSQ):
        for i in range(NSK):
            sub = keep_t[i * SK_T:(i + 1) * SK_T, j * SQ_T:(j + 1) * SQ_T]
            if not sub.any():
                cls[(i, j)] = "skip"
            elif sub.all():
                cls[(i, j)] = ("full", 0, SQ_T)
            else:
                # column bounding range with any unmasked entry
                cols = np.flatnonzero(sub.any(axis=0))
                cls[(i, j)] = (len(ptiles), int(cols[0]), int(cols[-1]) + 1)
                ptiles.append(np.ascontiguousarray(sub.astype(BF16)))
    return cls, ptiles


def _build(cls, n_ptiles):
    nc = bacc.Bacc("TRN2", target_bir_lowering=False, debug=False)
    dt = mybir.dt
    f32, bf = dt.float32, dt.bfloat16
    EXP = mybir.ActivationFunctionType.Exp
    NP_ = max(n_ptiles, 1)

    xq = nc.dram_tensor("xqt", [NSQ, 128, NCH, SQ_T], bf,
                        kind="ExternalInput").ap()
    xk = nc.dram_tensor("xkt", [NSQ, 128, NCH, SQ_T], bf,
                        kind="ExternalInput").ap()
    xv = nc.dram_tensor("xvt", [NSQ, 128, NCH, SQ_T], bf,
                        kind="ExternalInput").ap()
    wq = nc.dram_tensor("wqt", [128, NCH, FPG], bf, kind="ExternalInput").ap()
    wk = nc.dram_tensor("wkt", [128, NCH, FPG], bf, kind="ExternalInput").ap()
    wv = nc.dram_tensor("wvt", [128, NCH, FPG], bf, kind="ExternalInput").ap()
    wo = nc.dram_tensor("wot", [128, FPG // 128, D], bf,
                        kind="ExternalInput").ap()
    kp = nc.dram_tensor("keep", [128, NP_, SQ_T], bf,
                        kind="ExternalInput").ap()
    out = nc.dram_tensor("out", [S, D], bf, kind="ExternalOutput").ap()

    out_v = out.rearrange("(r p) o -> r p o", p=128)

    # per-j contiguous partial-tile ranges in the packed keep tensor
    kranges = []
    lo = 0
    for j in range(NSQ):
        hi = lo
        for i in range(NSK):
            cj = cls[(i, j)]
            if cj != "skip" and cj[0] != "full":
                hi = cj[0] + 1
        kranges.append((lo, max(hi, lo)))
        lo = max(hi, lo)

    with tile.TileContext(nc) as tc:
        with (
            tc.tile_pool(name="consts", bufs=1) as consts,
            tc.tile_pool(name="x", bufs=2) as xpool,
            tc.tile_pool(name="sc", bufs=2, space="PSUM") as sc_ps,
            tc.tile_pool(name="pv", bufs=2, space="PSUM") as pv_ps,
            tc.tile_pool(name="aux", bufs=2, space="PSUM") as aux_ps,
            tc.tile_pool(name="work", bufs=4) as work,
            tc.tile_pool(name="probs", bufs=8) as prpool,
        ):
            wq_sb = consts.tile([128, NCH, FPG], bf)
            wk_sb = consts.tile([128, NCH, FPG], bf)
            wv_sb = consts.tile([128, NCH, FPG], bf)
            wo_sb = consts.tile([128, FPG // 128, D], bf)
            keep_sb = consts.tile([128, NP_, SQ_T], bf)
            ones_sb = consts.tile([128, 128], bf)
            qh_sb = consts.tile([128, 2, S], bf)
            kh_sb = consts.tile([128, 2, S], bf)
            vh_sb = consts.tile([128, NSK, HPG, DH + 1], bf)
            att_sb = consts.tile([128, 2, S], bf)
            # per-(j%2, hp) normalization scratch; denominator rows 0 and 32
            l4 = [[consts.tile([128, SQ_T], f32, name=f"l4_{p}{h}")
                   for h in range(2)] for p in range(2)]
            r4f = [[consts.tile([128, SQ_T], f32, name=f"r4f_{p}{h}")
                    for h in range(2)] for p in range(2)]
            r4 = [[consts.tile([128, SQ_T], bf, name=f"r4_{p}{h}")
                   for h in range(2)] for p in range(2)]

            xtiles = {}
            pace_sb = consts.tile([1, 16], bf)

            def load_x(t):
                """Emit DMA triggers for iteration t's x tiles. For t=0 these
                go serially on sync in priority order; for t>0 the caller has
                placed a pacer on the gpsimd queue so the loads don't compete
                with the startup-critical transfers."""
                xq_t = xpool.tile([128, NCH, SQ_T], bf, tag="xq", name="xq_t")
                xk_t = xpool.tile([128, NCH, SQ_T], bf, tag="xk", name="xk_t")
                xv_t = xpool.tile([128, NCH, SQ_T], bf, tag="xv", name="xv_t")
                klo, khi = kranges[t]
                if t == 0:
                    # stage 1a: ONLY the q dependencies in flight, at full
                    # HBM bandwidth; k then v/keep/wo stages are released by
                    # pacers (load_stage1k / load_stage2) as q data lands
                    nc.sync.dma_start(wq_sb[:, 0:1, :], wq[:, 0:1, :])
                    nc.sync.dma_start(xq_t[:, 0:1, :], xq[t, :, 0:1, :])
                    nc.sync.dma_start(wq_sb[:, 1:2, :], wq[:, 1:2, :])
                    nc.sync.dma_start(xq_t[:, 1:2, :], xq[t, :, 1:2, :])
                    nc.sync.dma_start(wq_sb[:, 2:4, :], wq[:, 2:4, :])
                    nc.sync.dma_start(xq_t[:, 2:4, :], xq[t, :, 2:4, :])
                    nc.sync.dma_start(wq_sb[:, 4:8, :], wq[:, 4:8, :])
                    nc.sync.dma_start(xq_t[:, 4:8, :], xq[t, :, 4:8, :])
                    nc.gpsimd.memset(ones_sb[:], 1.0)
                    nc.gpsimd.memset(vh_sb[:], 1.0)  # ones col DH survives
                    for p in range(2):               # unused part. -> r = 1
                        nc.gpsimd.memset(l4[p][0][:], 1.0)
                        nc.gpsimd.memset(l4[p][1][:], 1.0)
                else:
                    nc.gpsimd.dma_start(xq_t[:], xq[t])
                    nc.gpsimd.dma_start(xk_t[:], xk[t])
                    nc.gpsimd.dma_start(xv_t[:], xv[t])
                    if khi > klo:
                        nc.gpsimd.dma_start(keep_sb[:, klo:khi, :],
                                            kp[:, klo:khi, :])
                xtiles[t] = (xq_t, xk_t, xv_t)

            def load_stage1k():
                """Release t=0's k loads once the q transfers have landed
                (pacer on the last xq piece), so q streams at full BW."""
                xk_t = xtiles[0][1]
                nc.gpsimd.tensor_copy(pace_sb[:], xtiles[0][0][0:1, 7, 0:16])
                nc.gpsimd.dma_start(wk_sb[:], wk[:])
                nc.gpsimd.dma_start(xk_t[:], xk[0])

            def load_stage2():
                """Release t=0's v/keep/wo loads once the first q-chain has
                evacuated (pacer), keeping startup bandwidth for q/k."""
                xv_t = xtiles[0][2]
                klo, khi = kranges[0]
                nc.gpsimd.tensor_copy(pace_sb[:], xtiles[0][0][0:1, 7, 0:16])
                nc.gpsimd.dma_start(wv_sb[:], wv[:])
                nc.gpsimd.dma_start(xv_t[:], xv[0])
                if khi > klo:
                    nc.gpsimd.dma_start(keep_sb[:, klo:khi, :],
                                        kp[:, klo:khi, :])
                nc.gpsimd.dma_start(wo_sb[:], wo[:])

            def proj_units(t):
                """Independent PE chain units for token-tile t's projections,
                in dependency order (q chains, k chains, v chains)."""
                xq_t, xk_t, xv_t = xtiles[t]
                sl = bass.ts(t, SQ_T)
                units = []
                for hout_, hp_ in ((qh_sb, 0), (qh_sb, 1), (kh_sb, 0),
                                   (kh_sb, 1)):
                    def u(hout=hout_, hp=hp_):
                        wsb = wq_sb if hout is qh_sb else wk_sb
                        xin = xq_t if hout is qh_sb else xk_t
                        hsl = bass.ts(hp, 128)
                        ps = aux_ps.tile([128, SQ_T], f32, tag="aux",
                                         name="ps")
                        for c in range(NCH):
                            nc.tensor.matmul(ps[:], wsb[:, c, hsl],
                                             xin[:, c, :], start=(c == 0),
                                             stop=(c == NCH - 1))
                        nc.vector.tensor_copy(hout[:, hp, sl], ps[:])
                    units.append(u)
                for s4_ in range(SQ_T // SK_T):
                    def u(s4=s4_):
                        i = t * (SQ_T // SK_T) + s4
                        psv = aux_ps.tile([128, SQ_T], f32, tag="aux",
                                          name="psv")
                        for c in range(NCH):
                            nc.tensor.matmul(psv[:, 0:FPG],
                                             xv_t[:, c, bass.ts(s4, SK_T)],
                                             wv_sb[:, c, :],
                                             start=(c == 0),
                                             stop=(c == NCH - 1))
                        nc.vector.tensor_copy(
                            vh_sb[:, i, :, 0:DH],
                            psv[:, 0:FPG].rearrange("p (h d) -> p h d",
                                                    h=HPG))
                    units.append(u)
                return units

            def attn_core(j, fill, fill_late=(), fill_after=(),
                          after_hp0=None):
                """Scores/exp/mask/PV for q-tile j, both head pairs, with
                independent `fill` units interleaved between a tile's scores
                and its PV so PE never waits on ScalarE exp. `fill_late` units
                are spread over the second (hp=1) half only; `after_hp0` runs
                right after hp=0's PSUM evacuation. Leaves unnormalized att in
                att_sb and denominators in l4[j % 2]."""
                jsl = bass.ts(j, SQ_T)
                kept = [i for i in range(NSK) if cls[(i, j)] != "skip"]
                ntiles = 2 * len(kept)
                fill = list(fill)
                # spread fill units evenly across the tile stream; late units
                # over the second half only
                fill_at = {}
                for f_i in range(len(fill)):
                    fill_at.setdefault(f_i * ntiles // max(len(fill), 1),
                                       []).append(fill[f_i])
                fill_late = list(fill_late)
                half = ntiles // 2
                for f_i in range(len(fill_late)):
                    pos = half + f_i * (ntiles - half) // max(len(fill_late),
                                                              1)
                    fill_at.setdefault(pos, []).append(fill_late[f_i])
                tile_no = 0
                for hp in range(2):
                    pv0 = pv_ps.tile([DH + 1, SQ_T], f32, tag="pv",
                                     name="pv0")
                    pv1 = pv_ps.tile([DH + 1, SQ_T], f32, tag="pv",
                                     name="pv1")

                    def tile_rng(n):
                        c, c0, c1 = cls[(kept[n], j)]
                        if n == 0:
                            c0, c1 = 0, SQ_T  # first tile must cover the bank
                        return c, c0, c1

                    def emit_scores(n):
                        c, c0, c1 = tile_rng(n)
                        isl = bass.ts(kept[n], SK_T)
                        qsl = bass.ds(j * SQ_T + c0, c1 - c0)
                        sc = sc_ps.tile([128, 2, SQ_T], f32, tag="sc",
                                        name="sc")
                        nc.tensor.matmul(sc[:, 0, c0:c1], kh_sb[0:64, hp, isl],
                                         qh_sb[0:64, hp, qsl], start=True,
                                         stop=True, tile_position=(0, 0))
                        nc.tensor.matmul(sc[:, 1, c0:c1],
                                         kh_sb[64:128, hp, isl],
                                         qh_sb[64:128, hp, qsl], start=True,
                                         stop=True, tile_position=(64, 0))
                        return sc

                    sc_n = emit_scores(0)
                    for n, i in enumerate(kept):
                        sc, (c, c0, c1) = sc_n, tile_rng(n)
                        if n + 1 < len(kept):
                            sc_n = emit_scores(n + 1)
                        for f in fill_at.pop(tile_no, ()):
                            f()
                        tile_no += 1
                        pr = prpool.tile([128, 2, SQ_T], bf, tag="probs",
                                         name="pr")
                        nc.scalar.activation(pr[:, :, c0:c1], sc[:, :, c0:c1],
                                             EXP, scale=0.125)
                        if c != "full":
                            nc.vector.tensor_mul(pr[:, 0, c0:c1],
                                                 pr[:, 0, c0:c1],
                                                 keep_sb[:, c, c0:c1])
                            nc.vector.tensor_mul(pr[:, 1, c0:c1],
                                                 pr[:, 1, c0:c1],
                                                 keep_sb[:, c, c0:c1])
                        nc.tensor.matmul(pv0[:, c0:c1],
                                         vh_sb[:, i, 2 * hp + 0, :],
                                         pr[:, 0, c0:c1], start=(n == 0),
                                         stop=(n == len(kept) - 1))
                        nc.tensor.matmul(pv1[:, c0:c1],
                                         vh_sb[:, i, 2 * hp + 1, :],
                                         pr[:, 1, c0:c1], start=(n == 0),
                                         stop=(n == len(kept) - 1))
                    # evacuate psum fast: unnormalized att + denominators.
                    # On the final head-pair the exp stream is finished, so
                    # the shift-free att block can evacuate on idle ScalarE
                    # and the denominator rows go first, shortening the tail
                    # DVE chain that gates the last normalization.
                    lj = l4[j % 2][hp]
                    if j == NSQ - 1 and hp == 1:
                        nc.vector.tensor_copy(lj[0:1, :], pv0[DH:DH + 1, :])
                        nc.scalar.copy(att_sb[0:64, hp, jsl], pv0[0:64, :])
                        nc.vector.tensor_copy(lj[32:33, :],
                                              pv1[DH:DH + 1, :])
                        nc.vector.tensor_copy(att_sb[64:128, hp, jsl],
                                              pv1[0:64, :])
                    else:
                        nc.vector.tensor_copy(lj[0:1, :], pv0[DH:DH + 1, :])
                        nc.vector.tensor_copy(att_sb[0:64, hp, jsl],
                                              pv0[0:64, :])
                        nc.vector.tensor_copy(lj[32:33, :],
                                              pv1[DH:DH + 1, :])
                        nc.vector.tensor_copy(att_sb[64:128, hp, jsl],
                                              pv1[0:64, :])
                    if hp == 0 and after_hp0 is not None:
                        after_hp0()
                for fs in fill_at.values():
                    for f in fs:
                        f()
                for f in fill_after:
                    f()

            def norm_hp_unit(j, hp, tail=False):
                """Normalization of q-tile j's head pair hp: r = 1/l on the
                pair's denominator rows, K=1 broadcast matmuls, in-place
                multiply of att_sb. In tail mode the shift-free copies run
                on ScalarE (idle once the exp stream ends)."""
                cp = nc.scalar.copy if tail else nc.vector.tensor_copy

                def u():
                    jsl = bass.ts(j, SQ_T)
                    lj = l4[j % 2][hp]
                    rjf = r4f[j % 2][hp]
                    rj = r4[j % 2][hp]
                    nc.vector.reciprocal_approx_fast(rjf[:], lj[:])
                    cp(rj[0:33, :], rjf[0:33, :])
                    rb_sb = work.tile([128, 2, SQ_T], bf, tag="rbsb",
                                      name="rb_sb")
                    rb = aux_ps.tile([128, SQ_T], f32, tag="aux", name="rb")
                    nc.tensor.matmul(rb[0:64, :], ones_sb[0:1, 0:64],
                                     rj[0:1, :], start=True,
                                     stop=True, tile_position=(0, 0))
                    nc.tensor.matmul(rb[64:128, :],
                                     ones_sb[32:33, 64:128],
                                     rj[32:33, :], start=True,
                                     stop=True, tile_position=(32, 64))
                    cp(rb_sb[:, hp, :], rb[:])
                    nc.vector.tensor_mul(att_sb[:, hp, jsl],
                                         att_sb[:, hp, jsl],
                                         rb_sb[:, hp, :])
                return u

            def outproj_units(j):
                """Row-parallel Wo for q-tile j as independent units (requires
                both of j's norm_hp units emitted first)."""
                units = []
                for t4_ in range(SQ_T // 128):
                    def u(t4=t4_):
                        r_ = j * (SQ_T // 128) + t4
                        tsl = bass.ds(j * SQ_T + t4 * 128, 128)
                        ost = work.tile([128, D], bf, tag="ost", name="ost")
                        for o in range(2):
                            po = aux_ps.tile([128, SQ_T], f32, tag="aux",
                                             name="po")
                            for hp in range(2):
                                nc.tensor.matmul(po[:], att_sb[:, hp, tsl],
                                                 wo_sb[:, hp,
                                                       bass.ts(o, 512)],
                                                 start=(hp == 0),
                                                 stop=(hp == 1))
                            nc.vector.tensor_copy(ost[:, bass.ts(o, 512)],
                                                  po[:])
                        nc.sync.dma_start(out_v[r_], ost[:])
                    units.append(u)
                return units

            # Window schedule: W0 = proj(0); W(t+1) = attn(t) interleaved
            # with proj(t+1) chains and norm/outproj(t-1) as fill. The last
            # window back-loads its fill, runs norm(hp=0) right after hp=0's
            # evacuation, and leaves only norm(hp=1) + outproj for the tail.
            load_x(0)
            load_stage1k()
            p0_units = proj_units(0)
            p0_units[0]()
            load_stage2()
            for u in p0_units[1:]:
                u()
            nc.gpsimd.tensor_copy(pace_sb[:], qh_sb[0:1, 0, bass.ds(0, 16)])
            load_x(1)
            last = NSQ - 1
            fills = {
                0: lambda: proj_units(1),
                1: lambda: proj_units(2) + [norm_hp_unit(0, 0),
                                            norm_hp_unit(0, 1)],
                2: lambda: (proj_units(3) + [norm_hp_unit(1, 0),
                                             norm_hp_unit(1, 1)]
                            + outproj_units(0)),
                3: lambda: ([norm_hp_unit(2, 0), norm_hp_unit(2, 1)]
                            + outproj_units(1) + outproj_units(2)),
            }
            for t in range(NSQ):
                fill = fills[t]()
                fill_late = [norm_hp_unit(t, 0)] if t == last else []
                attn_core(t, fill, fill_late, [])
                if t + 2 < NSQ:
                    # pacer: hold iteration t+2's loads off the DMA engines
                    # until iteration t+1's q-projection has landed, so
                    # startup-critical transfers get full HBM bandwidth
                    nc.gpsimd.tensor_copy(
                        pace_sb[:],
                        qh_sb[0:1, 0, bass.ds((t + 1) * SQ_T, 16)])
                    load_x(t + 2)
            norm_hp_unit(last, 1, tail=True)()
            for u in outproj_units(last):
                u()

    nc.compile()
    return nc


def _get_nc(mask):
    key = hash(np.asarray(mask, dtype=bool).tobytes())
    if key not in _BUILT:
        cls, ptiles = _classify(mask)
        _BUILT[key] = (_build(cls, len(ptiles)), cls, ptiles)
    return _BUILT[key]


def _tile_x(x):
    """[S, D] f32 -> [NSQ, 128, NCH, SQ_T] bf16, contiguous per partition."""
    xt = x.T.astype(BF16)                      # [D, S]
    xt = xt.reshape(NCH, 128, NSQ, SQ_T)       # [c, p, t, s]
    return np.ascontiguousarray(xt.transpose(2, 1, 0, 3))


def _tile_w(w):
    """[FPG, D] slice (already W[fsl,:].T = [D, FPG]) -> [128, NCH, FPG]."""
    return np.ascontiguousarray(w.reshape(NCH, 128, FPG).transpose(1, 0, 2))


def _kernel_impl(q, k, v, attn_mask, Wq, Wk, Wv, Wo, trace=False):
    q = np.asarray(q, dtype=np.float32)
    k = np.asarray(k, dtype=np.float32)
    v = np.asarray(v, dtype=np.float32)
    nc, cls, ptiles = _get_nc(attn_mask)

    if ptiles:
        keep_packed = np.stack(ptiles, axis=0)          # [n, 128, 512]
    else:
        keep_packed = np.zeros((1, SK_T, SQ_T), dtype=BF16)
    keep_packed = np.ascontiguousarray(keep_packed.transpose(1, 0, 2))

    xt = {}
    for b in range(B):
        xt[("q", b)] = _tile_x(q[b])
        xt[("k", b)] = _tile_x(k[b])
        xt[("v", b)] = _tile_x(v[b])
    wslices = {}
    for g in range(GROUPS):
        fsl = slice(g * FPG, (g + 1) * FPG)
        wslices[("wq", g)] = _tile_w(Wq[fsl, :].T.astype(BF16))
        wslices[("wk", g)] = _tile_w(Wk[fsl, :].T.astype(BF16))
        wslices[("wv", g)] = _tile_w(Wv[fsl, :].T.astype(BF16))
        wot = Wo[:, fsl].T.astype(BF16)                 # [FPG, D]
        wslices[("wo", g)] = np.ascontiguousarray(
            wot.reshape(2, 128, D).transpose(1, 0, 2))
    in_maps = []
    for core in range(NCORES):
        b, g = core // GROUPS, core % GROUPS
        in_maps.append({
            "xqt": xt[("q", b)], "xkt": xt[("k", b)], "xvt": xt[("v", b)],
            "wqt": wslices[("wq", g)], "wkt": wslices[("wk", g)],
            "wvt": wslices[("wv", g)], "wot": wslices[("wo", g)],
            "keep": keep_packed,
        })

    res = bass_utils.run_bass_kernel_spmd(
        nc, in_maps, core_ids=list(range(NCORES)), trace=trace)

    out = np.zeros((B, S, D), dtype=np.float32)
    for core in range(NCORES):
        out[core // GROUPS] += res.results[core]["out"].astype(np.float32)
    return out, res


def kernel(q, k, v, attn_mask, Wq, Wk, Wv, Wo):
    out, _ = _kernel_impl(q, k, v, attn_mask, Wq, Wk, Wv, Wo)
    return out
